# revision 1
# baseline (speedup 1.0000x reference)
"""MixedQLinear Trainium2 kernel.

Computation (per reference):
  x2 = x[0]                                  (M=4096, IN_F=4096) fp16
  int_x = x2[:, int_indices]                 (M, 3840)
  fp_x  = x2[:, fp_indices]                  (M, 256)
  per-token asym quant of int_x to int4:  scale=(mx-mn)/15, zero=mn
  q = round((int_x-zero)/scale) - 8          in [-8,7]
  out = scale*w_scale*(q @ w_int.T) + (zero+8*scale)*reduced_w + fp_x@fp_w.T + bias

Strategy: tensor-parallel over out_features (512 per core, 8 cores).
x replicated; per-token quantization replicated on every core.

Device-side per core:
  - per 128-token tile: min/max stats (DVE tensor_tensor_reduce),
    quantize via ScalarE activation (per-partition scale/bias) + fp32
    magic-constant RNE rounding on DVE,
  - alpha = mn/scale + 8 appended as an extra "activation" column so the
    (zero + 8*scale)*reduced_w term rides the int matmul (weight row =
    reduced_w),
  - DMA xbar transpose (128 x 3968) -> k-major q tiles,
  - PE matmul q @ (w_int*w_scale) accumulating K=3968 (31 steps), plus a
    separate K=384 fp16 matmul for the fp path (ones row handles bias),
  - combine: out = psum_int * scale_t + psum_fp  (single DVE op).

Host side does only layout work: column gather, int4 unpack, weight
scaling/reshaping, sharding, and concat of per-core outputs.
"""

import os
import sys

import numpy as np

for _p in ("/opt/trn_rl_repo",):
    if _p not in sys.path and os.path.isdir(_p):
        sys.path.insert(0, _p)

TOKENS = 4096
IN_F = 4096
OUT_F = 4096
FP_F = 256
INT_F = IN_F - FP_F          # 3840
NCORES = 8
OPC = OUT_F // NCORES        # 512 out features per core
KP = INT_F + 128             # 3968 = 31*128 (alpha col + zero pad)
KE = KP // 128               # 31 k-tiles
NT = TOKENS // 128           # 32 token tiles
C_MAGIC = 12582912.0         # 1.5*2^23: fp32 add/sub forces RNE-to-integer

_PROGRAM = None
LAST_RESULTS = None


def _ensure_ntff_hook():
    """Install the axon NTFF profiling hook if the image's antenv lacks it.

    Best-effort: profiling only; compile/run work without it.
    """
    import contextlib
    import ctypes
    import types

    try:
        try:
            import antenv.axon_hooks as hooks_mod
        except ImportError:
            import antenv

            hooks_mod = types.ModuleType("antenv.axon_hooks")
            _holder = {}
            hooks_mod.set_axon_ntff_profile_hook = (
                lambda hook: _holder.__setitem__("hook", hook))
            hooks_mod.get_axon_ntff_profile_hook = (
                lambda: _holder.get("hook"))
            sys.modules["antenv.axon_hooks"] = hooks_mod
            antenv.axon_hooks = hooks_mod

        if hooks_mod.get_axon_ntff_profile_hook() is not None:
            return
        so_path = "/opt/axon/libaxon_pjrt.so"
        if not os.path.exists(so_path):
            return
        lib = ctypes.CDLL(so_path)
        if not hasattr(lib, "axon_start_nrt_profile"):
            return
        lib.axon_start_nrt_profile.argtypes = [
            ctypes.POINTER(ctypes.c_int64), ctypes.c_size_t]
        lib.axon_start_nrt_profile.restype = ctypes.c_int64
        lib.axon_stop_nrt_profile.argtypes = [ctypes.c_char_p]
        lib.axon_stop_nrt_profile.restype = ctypes.c_int64

        @contextlib.contextmanager
        def _hook(output_dir, device_ids):
            import jax

            jax.devices()
            if device_ids:
                ids = (ctypes.c_int64 * len(device_ids))(*device_ids)
                rc = lib.axon_start_nrt_profile(ids, len(device_ids))
            else:
                rc = lib.axon_start_nrt_profile(None, 0)
            if rc != 0:
                raise RuntimeError(f"axon_start_nrt_profile rc={rc}")
            try:
                yield
            finally:
                n = lib.axon_stop_nrt_profile(str(output_dir).encode())
                print(f"ntff profile: {n} file(s) written to {output_dir}")

        hooks_mod.set_axon_ntff_profile_hook(_hook)
    except Exception:
        pass


def _build_program():
    import concourse.mybir as mybir
    import concourse.tile as tile
    from concourse import bacc

    f16 = mybir.dt.float16
    f32 = mybir.dt.float32
    f8 = mybir.dt.float8e4
    Alu = mybir.AluOpType

    nc = bacc.Bacc(None, target_bir_lowering=False)

    x_int = nc.dram_tensor("x_int", [TOKENS, INT_F], f16, kind="ExternalInput")
    x_st = nc.dram_tensor("x_st", [TOKENS // NCORES, INT_F], f16,
                          kind="ExternalInput")
    fpx_d = nc.dram_tensor("fpx", [128, 3, TOKENS], f16, kind="ExternalInput")
    # raw int4 weight values (exact in fp8e4m3), k-major: [p, e, o] =
    # w[k = e*128+p, o]; wscale applied in the combine via wsb
    wq_d = nc.dram_tensor("wq", [128, KE - 1, OPC], f8, kind="ExternalInput")
    wsb_d = nc.dram_tensor("wsb", [128, OPC], f32, kind="ExternalInput")
    redw_d = nc.dram_tensor("redw", [1, OPC], f16, kind="ExternalInput")
    fpw_d = nc.dram_tensor("fpw", [128, 3, OPC], f16, kind="ExternalInput")
    out_d = nc.dram_tensor("out", [TOKENS, OPC], f16, kind="ExternalOutput")
    TPC = TOKENS // NCORES // 128     # own token tiles per core (4)

    with tile.TileContext(nc) as tc:
        with tc.tile_pool(name="consts", bufs=1) as consts, \
             tc.tile_pool(name="xin", bufs=3) as xin, \
             tc.tile_pool(name="xsp", bufs=2) as xsp, \
             tc.tile_pool(name="y0p", bufs=2) as y0p, \
             tc.tile_pool(name="qap", bufs=2) as qap, \
             tc.tile_pool(name="qtp", bufs=4) as qtp, \
             tc.tile_pool(name="qt8", bufs=4) as qt8, \
             tc.tile_pool(name="jnk", bufs=2) as jnk, \
             tc.tile_pool(name="stp", bufs=4) as stp, \
             tc.tile_pool(name="outp", bufs=3) as outp, \
             tc.tile_pool(name="dram", bufs=1, space="DRAM") as dram, \
             tc.tile_pool(name="ps0", bufs=4, space="PSUM") as ps0, \
             tc.tile_pool(name="ps1", bufs=3, space="PSUM") as ps1:

            # === Phase A: per-token quant params for this core's own
            # tokens (TPC tiles), then AllGather so every core has all
            # 4096 tokens' params.  Packed per tile r as 4 fp32 columns:
            # [scale, rs, bq, alpha] at params[:, 4r:4r+4].
            ppack = consts.tile([128, 4 * TPC], f32)
            for r in range(TPC):
                xst = xsp.tile([128, INT_F], f16)
                nc.gpsimd.dma_start(
                    out=xst[:, :], in_=x_st[r * 128:(r + 1) * 128, :])
                mn = stp.tile([128, 1], f32)
                mx = stp.tile([128, 1], f32)
                a1 = jnk.tile([128, 1920], f16, tag="a1")
                a2 = jnk.tile([128, 960], f16, tag="a2")
                nc.vector.tensor_tensor(
                    out=a1[:, :], in0=xst[:, :1920], in1=xst[:, 1920:],
                    op=Alu.min)
                nc.vector.tensor_tensor(
                    out=a2[:, :], in0=a1[:, :960], in1=a1[:, 960:], op=Alu.min)
                nc.vector.tensor_reduce(
                    out=mn[:, :], in_=a2[:, :], axis=mybir.AxisListType.X,
                    op=Alu.min)
                b1 = jnk.tile([128, 1920], f16, tag="b1")
                b2 = jnk.tile([128, 960], f16, tag="b2")
                nc.vector.tensor_tensor(
                    out=b1[:, :], in0=xst[:, :1920], in1=xst[:, 1920:],
                    op=Alu.max)
                nc.vector.tensor_tensor(
                    out=b2[:, :], in0=b1[:, :960], in1=b1[:, 960:], op=Alu.max)
                nc.vector.tensor_reduce(
                    out=mx[:, :], in_=b2[:, :], axis=mybir.AxisListType.X,
                    op=Alu.max)
                d = stp.tile([128, 1], f32)
                nc.vector.tensor_sub(d[:, :], mx[:, :], mn[:, :])
                nc.vector.tensor_scalar(
                    out=ppack[:, 4 * r:4 * r + 1], in0=d[:, :],
                    scalar1=1.0 / 15.0, scalar2=1e-8, op0=Alu.mult, op1=Alu.max)
                nc.vector.reciprocal(
                    ppack[:, 4 * r + 1:4 * r + 2], ppack[:, 4 * r:4 * r + 1])
                tt = stp.tile([128, 1], f32)
                nc.vector.tensor_mul(
                    tt[:, :], mn[:, :], ppack[:, 4 * r + 1:4 * r + 2])
                nc.vector.tensor_scalar(
                    out=ppack[:, 4 * r + 2:4 * r + 3], in0=tt[:, :],
                    scalar1=-1.0, scalar2=-8.0, op0=Alu.mult, op1=Alu.add)
                # alpha' = mn + 8*scale  (the zero-point term, pre-scaled:
                # it is added via the fp-path matmul which is NOT
                # multiplied by scale_t in the combine)
                t8 = stp.tile([128, 1], f32)
                nc.vector.tensor_scalar(
                    out=t8[:, :], in0=ppack[:, 4 * r:4 * r + 1],
                    scalar1=8.0, scalar2=None, op0=Alu.mult)
                nc.vector.tensor_add(
                    ppack[:, 4 * r + 3:4 * r + 4], t8[:, :], mn[:, :])

            # const loads ride the gpsimd queue right after the stats
            # inputs so the small stats loads hit SDMA first
            wq_s = consts.tile([128, KE - 1, OPC], f8)
            nc.gpsimd.dma_start(out=wq_s[:, :, :], in_=wq_d[:, :, :])
            fpx_s = consts.tile([128, 3, TOKENS], f16)
            nc.gpsimd.dma_start(out=fpx_s[:, :, :], in_=fpx_d[:, :, :])
            fpw_s = consts.tile([128, 3, OPC], f16)
            nc.gpsimd.dma_start(out=fpw_s[:, :, :], in_=fpw_d[:, :, :])
            wsb_s = consts.tile([128, OPC], f32)
            nc.gpsimd.dma_start(out=wsb_s[:, :], in_=wsb_d[:, :])
            redw_s = consts.tile([1, OPC], f16)
            nc.gpsimd.dma_start(out=redw_s[:, :], in_=redw_d[:, :])

            # x-loads for the first LA tiles dispatch before the bounce
            # DMA (which blocks the gpsimd FIFO until stats finish)
            preloaded = {}
            for j in range(3):
                xt = xin.tile([128, INT_F], f16)
                nc.gpsimd.dma_start(
                    out=xt[:, :], in_=x_int[j * 128:(j + 1) * 128, :])
                preloaded[j] = xt

            in_b = dram.tile([128, 4 * TPC], f32)
            out_b = dram.tile([NCORES * 128, 4 * TPC], f32)
            nc.gpsimd.dma_start(out=in_b[:, :], in_=ppack[:, :])
            nc.gpsimd.collective_compute(
                "AllGather",
                Alu.bypass,
                replica_groups=[list(range(NCORES))],
                ins=[in_b.opt()],
                outs=[out_b.opt()],
            )
            # pall[p, c*4*TPC + w] = params of core c, pack col w
            pall = consts.tile([128, NCORES * 4 * TPC], f32)
            nc.gpsimd.dma_start(
                out=pall[:, :].rearrange("p (c w) -> p c w", c=NCORES),
                in_=out_b[:, :].rearrange("(c p) w -> p c w", c=NCORES))

            def param(j, v):
                """AP of param v for token tile j: [scale, rs, bq, alpha]."""
                idx = (j // TPC) * 4 * TPC + (j % TPC) * 4 + v
                return pall[:, idx:idx + 1]

            # === Phase B: software-pipelined main loop.  Producer ops
            # (load/quantize/round/transpose) for tile j+LA are emitted
            # before consumer ops (matmuls/combine/store) for tile j so
            # each engine's FIFO never blocks the producers on a
            # consumer-side dependency.
            def producer(j):
                if j in preloaded:
                    xt = preloaded.pop(j)
                else:
                    xt = xin.tile([128, INT_F], f16)
                    # NOT on nc.sync: the sync FIFO is dedicated to the
                    # blocking DMA transposes; a load queued behind one
                    # would serialize the whole producer pipeline.
                    nc.gpsimd.dma_start(
                        out=xt[:, :], in_=x_int[j * 128:(j + 1) * 128, :])
                # quantize: y0 = x*rs + bq (fp32), q = (y0+C)-C (RNE round)
                y0 = y0p.tile([128, INT_F], f32)
                nc.scalar.activation(
                    out=y0[:, :], in_=xt[:, :],
                    func=mybir.ActivationFunctionType.Identity,
                    bias=param(j, 2), scale=param(j, 1))
                qa = qap.tile([128, KP], f16)
                nc.vector.tensor_scalar(
                    out=qa[:, :INT_F], in0=y0[:, :], scalar1=C_MAGIC,
                    scalar2=-C_MAGIC, op0=Alu.add, op1=Alu.add)
                # alpha column (mn*rs + 8) and zero padding
                nc.vector.tensor_copy(
                    out=qa[:, INT_F:INT_F + 1], in_=param(j, 3))
                nc.vector.memset(qa[:, INT_F + 1:], 0.0)
                # k-major transpose via DMA xbar: qtb[p,e,t] = qa[t,e*128+p]
                qtb = qtp.tile([128, KE, 128], f16)
                nc.sync.dma_start_transpose(out=qtb[:, :, :], in_=qa[:, :])
                # fp8 copy of the q part for the DoubleRow matmul
                # (split DVE/ACT per tile parity; gpsimd is far too slow
                # at this and its SBUF port contends with DVE)
                q8 = qt8.tile([128, KE - 1, 128], f8)
                if j % 2 == 0:
                    nc.vector.tensor_copy(out=q8[:, :, :], in_=qtb[:, :KE - 1, :])
                else:
                    nc.scalar.copy(q8[:, :, :], qtb[:, :KE - 1, :])
                return qtb, q8

            def consumer(j, tiles):
                qtb, q8 = tiles
                t0 = j * 128
                # int matmul: fp8 DoubleRow, contraction 256/step, q is the
                # stationary (2 int4 values per PE cell), raw-int weights
                # are the moving operand
                p0 = ps0.tile([128, OPC], f32)
                if os.environ.get("NO_DR"):
                    for e in range(KE - 1):
                        nc.tensor.matmul(
                            p0[:, :], q8[:, e, :], wq_s[:, e, :],
                            start=(e == 0), stop=(e == KE - 2))
                else:
                    for e in range(15):
                        nc.tensor.matmul(
                            p0[:, :], q8[:, 2 * e:2 * e + 2, :],
                            wq_s[:, 2 * e:2 * e + 2, :],
                            start=(e == 0), stop=(e == 14),
                            perf_mode=mybir.MatmulPerfMode.DoubleRow)
                # fp path + alpha*reduced_w via the transposed alpha row
                p1 = ps1.tile([128, OPC], f32)
                for c in range(3):
                    nc.tensor.matmul(
                        p1[:, :], fpx_s[:, c, t0:t0 + 128], fpw_s[:, c, :],
                        start=(c == 0), stop=False)
                nc.tensor.matmul(
                    p1[:, :], qtb[0:1, KE - 1, :], redw_s[0:1, :],
                    start=False, stop=True)
                # combine: out = (p0*wscale)*scale_t + p1
                m = outp.tile([128, OPC], f32, tag="s1")
                nc.vector.tensor_mul(m[:, :], p0[:, :], wsb_s[:, :])
                ot = outp.tile([128, OPC], f16, tag="ot")
                nc.vector.affine_then_add(
                    out=ot[:, :], in0=m[:, :], in1=p1[:, :],
                    scale=param(j, 0), bias=0.0)
                nc.gpsimd.dma_start(out=out_d[t0:t0 + 128, :], in_=ot[:, :])

            LA = 3
            made = {}
            for j in range(NT):
                made[j] = producer(j)
                if j >= LA:
                    consumer(j - LA, made.pop(j - LA))
            for j in range(NT - LA, NT):
                consumer(j, made.pop(j))

    nc.finalize()
    return nc


def _get_program():
    global _PROGRAM
    if _PROGRAM is None:
        _PROGRAM = _build_program()
    return _PROGRAM


def _unpack_i4(w_packed):
    """(out, INT_F//2) uint8 -> (out, INT_F) int8; col 2k=low nibble, 2k+1=high."""
    lo = (w_packed & 0x0F).astype(np.int8)
    hi = ((w_packed >> 4) & 0x0F).astype(np.int8)
    lo = np.where(lo >= 8, lo - 16, lo)
    hi = np.where(hi >= 8, hi - 16, hi)
    w = np.empty((w_packed.shape[0], w_packed.shape[1] * 2), dtype=np.int8)
    w[:, 0::2] = lo
    w[:, 1::2] = hi
    return w


def _prep_inputs(x, int_weight, weights_scales, reduced_w, fp_weight, bias,
                 int_indices, fp_indices):
    x2 = np.asarray(x, dtype=np.float16)[0]
    int_idx = np.asarray(int_indices).astype(np.int64)
    fp_idx = np.asarray(fp_indices).astype(np.int64)

    x_int = np.ascontiguousarray(x2[:, int_idx])            # (M, 3840) f16

    fp_xT = np.ascontiguousarray(x2[:, fp_idx].T)           # (256, M) f16
    fpx = np.zeros((128, 3, TOKENS), dtype=np.float16)
    fpx[:, 0, :] = fp_xT[:128]
    fpx[:, 1, :] = fp_xT[128:256]
    fpx[0, 2, :] = 1.0                                      # ones row (bias)

    w_int = _unpack_i4(np.asarray(int_weight))              # (OUT_F, 3840) int8
    wsc = np.asarray(weights_scales).astype(np.float32)     # (OUT_F, 1)
    redw = np.asarray(reduced_w).astype(np.float16)         # (1, OUT_F)
    fpW = np.asarray(fp_weight).astype(np.float16)          # (OUT_F, 256)
    b = np.asarray(bias).astype(np.float16)                 # (OUT_F,)

    import ml_dtypes
    f8np = ml_dtypes.float8_e4m3

    in_maps = []
    for c in range(NCORES):
        rows = slice(c * OPC, (c + 1) * OPC)
        # raw int4 weights, k-major, fp8 (exact); wscale goes to wsb
        wq = np.ascontiguousarray(
            w_int[rows].T.reshape(KE - 1, 128, OPC).transpose(1, 0, 2)
        ).astype(f8np)
        wsb = np.broadcast_to(
            wsc[rows, 0][None, :], (128, OPC)).astype(np.float32).copy()
        redw_row = redw[0, rows][None, :].copy()                # (1, OPC) f16

        fpw_aug = np.zeros((384, OPC), dtype=np.float16)
        fpw_aug[:FP_F] = fpW[rows].T
        fpw_aug[FP_F] = b[rows]
        fpw = np.ascontiguousarray(
            fpw_aug.reshape(3, 128, OPC).transpose(1, 0, 2))

        tpc = TOKENS // NCORES
        x_st = np.ascontiguousarray(x_int[c * tpc:(c + 1) * tpc])
        in_maps.append({"x_int": x_int, "x_st": x_st, "fpx": fpx,
                        "wq": wq, "wsb": wsb, "redw": redw_row, "fpw": fpw})
    return in_maps


def kernel(x, int_weight, weights_scales, reduced_w, fp_weight, bias,
           int_indices, fp_indices):
    global LAST_RESULTS
    from concourse.bass_utils import run_bass_kernel_spmd

    _ensure_ntff_hook()
    in_maps = _prep_inputs(x, int_weight, weights_scales, reduced_w,
                           fp_weight, bias, int_indices, fp_indices)
    nc = _get_program()
    res = run_bass_kernel_spmd(nc, in_maps, core_ids=list(range(NCORES)))
    LAST_RESULTS = res
    out = np.concatenate([res.results[c]["out"] for c in range(NCORES)], axis=1)
    return out[None].astype(np.float16)



# revision 9
# speedup vs baseline: 1.8404x; 1.8404x over previous
"""MixedQLinear Trainium2 kernel — token-sharded (data-parallel) version.

Computation (per reference):
  x2 = x[0]                                  (M=4096, IN_F=4096) fp16
  int_x = x2[:, int_indices]                 (M, 3840)
  fp_x  = x2[:, fp_indices]                  (M, 256)
  per-token asym quant of int_x to int4:  scale=(mx-mn)/15, zero=mn
  q = round((int_x-zero)/scale) - 8          in [-8,7]
  out = scale*w_scale*(q @ w_int.T) + (zero+8*scale)*reduced_w + fp_x@fp_w.T + bias

Strategy: shard TOKENS across the 8 cores (512 each); every core holds the
FULL weight set in SBUF (int4 weights as fp8 ~15.7MB, fp path fp8, scales).
Each core quantizes only its own 4 token tiles — 8x less DVE/ACT/transpose
work than out-feature sharding, and no collective at all.

Device-side per core, per 128-token tile:
  - min/max stats (DVE trees, scratch carved from the ya/qtb buffers),
  - ACT emits ya = x*rs + (bq+1536) in f16: the f16 output cast IS the
    round-to-nearest-int (f16 ulp=1 on [1024,2048)), so no separate
    rounding pass; alpha = mn+8*scale rides as an extra column,
  - DMA xbar transpose (128 x 3968) -> k-major,
  - the -1536 de-bias is folded into the f16->fp8 copy (tensor_scalar),
  - per 512-wide out chunk (8 chunks): 15 fp8 DoubleRow matmuls (int path,
    K=3840) into p0; 2 fp8 DR matmuls (fp path, K=512: 256 fp features +
    ones-row bias + pad) plus the fp16 alpha x reduced_w rank-1 into p1,
  - combine: out = (p0*wscale)*scale_t + p1  (DVE mul + affine_then_add).

Host side: column gather, int4 unpack, fp8 casts, k-major weight layout.
"""

import os
import sys

import numpy as np

for _p in ("/opt/trn_rl_repo",):
    if _p not in sys.path and os.path.isdir(_p):
        sys.path.insert(0, _p)

TOKENS = 4096
IN_F = 4096
OUT_F = 4096
FP_F = 256
INT_F = IN_F - FP_F          # 3840
NCORES = 8
TPC = TOKENS // NCORES       # 512 tokens per core
NT = TPC // 128              # 4 token tiles per core
KE = INT_F // 128            # 30 int k-tiles
NCH = 8                      # out chunks
CHW = OUT_F // NCH           # 512 outs per chunk
KPF = 512                    # padded fp-path contraction (256 fp + bias + pad)
C16 = 1536.0                 # f16 magic: cast of (v+1536) to f16 rounds v to int

_PROGRAM = None
LAST_RESULTS = None


def _ensure_ntff_hook():
    """Install the axon NTFF profiling hook if the image's antenv lacks it.

    Best-effort: profiling only; compile/run work without it.
    """
    import contextlib
    import ctypes
    import types

    try:
        try:
            import antenv.axon_hooks as hooks_mod
        except ImportError:
            import antenv

            hooks_mod = types.ModuleType("antenv.axon_hooks")
            _holder = {}
            hooks_mod.set_axon_ntff_profile_hook = (
                lambda hook: _holder.__setitem__("hook", hook))
            hooks_mod.get_axon_ntff_profile_hook = (
                lambda: _holder.get("hook"))
            sys.modules["antenv.axon_hooks"] = hooks_mod
            antenv.axon_hooks = hooks_mod

        if hooks_mod.get_axon_ntff_profile_hook() is not None:
            return
        so_path = "/opt/axon/libaxon_pjrt.so"
        if not os.path.exists(so_path):
            return
        lib = ctypes.CDLL(so_path)
        if not hasattr(lib, "axon_start_nrt_profile"):
            return
        lib.axon_start_nrt_profile.argtypes = [
            ctypes.POINTER(ctypes.c_int64), ctypes.c_size_t]
        lib.axon_start_nrt_profile.restype = ctypes.c_int64
        lib.axon_stop_nrt_profile.argtypes = [ctypes.c_char_p]
        lib.axon_stop_nrt_profile.restype = ctypes.c_int64

        @contextlib.contextmanager
        def _hook(output_dir, device_ids):
            import jax

            jax.devices()
            if device_ids:
                ids = (ctypes.c_int64 * len(device_ids))(*device_ids)
                rc = lib.axon_start_nrt_profile(ids, len(device_ids))
            else:
                rc = lib.axon_start_nrt_profile(None, 0)
            if rc != 0:
                raise RuntimeError(f"axon_start_nrt_profile rc={rc}")
            try:
                yield
            finally:
                n = lib.axon_stop_nrt_profile(str(output_dir).encode())
                print(f"ntff profile: {n} file(s) written to {output_dir}")

        hooks_mod.set_axon_ntff_profile_hook(_hook)
    except Exception:
        pass


def _build_program():
    import concourse.mybir as mybir
    import concourse.tile as tile
    from concourse import bacc

    f16 = mybir.dt.float16
    f32 = mybir.dt.float32
    f8 = mybir.dt.float8e4
    Alu = mybir.AluOpType
    DR = mybir.MatmulPerfMode.DoubleRow

    nc = bacc.Bacc(None, target_bir_lowering=False)

    x_own = nc.dram_tensor("x_own", [TPC, INT_F], f16, kind="ExternalInput")
    fpx8_d = nc.dram_tensor("fpx8", [128, 2, 2, TPC], f8, kind="ExternalInput")
    wq_d = nc.dram_tensor("wq", [128, NCH, KE, CHW], f8, kind="ExternalInput")
    fpw8_d = nc.dram_tensor("fpw8", [128, 2, 2, OUT_F], f8,
                            kind="ExternalInput")
    wsb_d = nc.dram_tensor("wsb", [128, OUT_F], f16, kind="ExternalInput")
    redw_d = nc.dram_tensor("redw", [1, OUT_F], f16, kind="ExternalInput")
    out_d = nc.dram_tensor("out", [TPC, OUT_F], f16, kind="ExternalOutput")

    with tile.TileContext(nc) as tc:
        with tc.tile_pool(name="consts", bufs=1) as consts, \
             tc.tile_pool(name="xin", bufs=1) as xin, \
             tc.tile_pool(name="qt8", bufs=2) as qt8, \
             tc.tile_pool(name="stp", bufs=4) as stp, \
             tc.tile_pool(name="outp", bufs=2) as outp, \
             tc.tile_pool(name="mp", bufs=1) as mp, \
             tc.tile_pool(name="ps0", bufs=3, space="PSUM") as ps0, \
             tc.tile_pool(name="ps1", bufs=3, space="PSUM") as ps1:

            # First x tile rides the DMA queues ahead of the big weight load.
            xt0 = xin.tile([128, INT_F], f16, tag="xt")
            nc.gpsimd.dma_start(out=xt0[:, :], in_=x_own[0:128, :])

            fpx8_s = consts.tile([128, 2, 2, TPC], f8)
            nc.gpsimd.dma_start(out=fpx8_s[:, :, :, :], in_=fpx8_d[:, :, :, :])
            wsb_s = consts.tile([128, OUT_F], f16)
            nc.gpsimd.dma_start(out=wsb_s[:, :], in_=wsb_d[:, :])
            redw_s = consts.tile([1, OUT_F], f16)
            nc.gpsimd.dma_start(out=redw_s[:, :], in_=redw_d[:, :])
            fpw8_s = consts.tile([128, 2, 2, OUT_F], f8)
            nc.gpsimd.dma_start(out=fpw8_s[:, :, :, :], in_=fpw8_d[:, :, :, :])
            # wq in priority order: 16 sub-DMAs per chunk, one per HW DMA
            # queue, so the queues collectively finish chunk c before
            # chunk c+1 (a single 8-way concurrent load would deliver all
            # chunks at ~60us; the PE needs chunk 0 at ~15us).
            wq_s = consts.tile([128, NCH, KE, CHW], f8)
            wq_sf = wq_s.rearrange("p c e o -> p c (e o)")
            wq_df = wq_d[:, :, :, :].rearrange("p c e o -> p c (e o)")
            csz = KE * CHW
            nsub = 16
            for c in range(NCH):
                for j in range(nsub):
                    s0 = j * (csz // nsub)
                    s1 = (j + 1) * (csz // nsub)
                    nc.gpsimd.dma_start(
                        out=wq_sf[:, c, s0:s1], in_=wq_df[:, c, s0:s1])

            # Manual double-buffers for ya (pre-transpose, biased quant) and
            # qtb (post-transpose) so the pad columns can be zeroed once.
            ya_b = [consts.tile([128, KE * 128 + 128], f16, name=f"ya{i}",
                                tag=f"ya{i}") for i in range(2)]
            qtb_b = [consts.tile([128, (KE + 1) * 128], f16, name=f"qt{i}",
                                 tag=f"qt{i}") for i in range(2)]
            for i in range(2):
                nc.vector.memset(ya_b[i][:, INT_F + 1:], 0.0)

            ppack = consts.tile([128, 4 * NT], f32)
            negc = consts.tile([128, 1], f32)
            nc.vector.memset(negc[:, :], -C16)
            onec = consts.tile([128, 1], f32)
            nc.vector.memset(onec[:, :], 1.0)

            def producer(r, xt):
                ya = ya_b[r % 2]
                qtb = qtb_b[r % 2]
                # min tree (scratch carved from ya; overwritten by ACT after)
                mn = stp.tile([128, 1], f32, tag="mn")
                mx = stp.tile([128, 1], f32, tag="mx")
                nc.vector.tensor_tensor(
                    out=ya[:, :1920], in0=xt[:, :1920], in1=xt[:, 1920:],
                    op=Alu.min)
                nc.vector.tensor_tensor(
                    out=ya[:, 1920:2880], in0=ya[:, :960], in1=ya[:, 960:1920],
                    op=Alu.min)
                nc.vector.tensor_reduce(
                    out=mn[:, :], in_=ya[:, 1920:2880],
                    axis=mybir.AxisListType.X, op=Alu.min)
                # max tree (scratch carved from qtb; overwritten by transpose)
                nc.vector.tensor_tensor(
                    out=qtb[:, :1920], in0=xt[:, :1920], in1=xt[:, 1920:],
                    op=Alu.max)
                nc.vector.tensor_tensor(
                    out=qtb[:, 1920:2880], in0=qtb[:, :960],
                    in1=qtb[:, 960:1920], op=Alu.max)
                nc.vector.tensor_reduce(
                    out=mx[:, :], in_=qtb[:, 1920:2880],
                    axis=mybir.AxisListType.X, op=Alu.max)
                # params: [scale, rs, bq+1536, alpha] at ppack[:, 4r:4r+4]
                d = stp.tile([128, 1], f32, tag="d")
                nc.vector.tensor_sub(d[:, :], mx[:, :], mn[:, :])
                nc.vector.tensor_scalar(
                    out=ppack[:, 4 * r:4 * r + 1], in0=d[:, :],
                    scalar1=1.0 / 15.0, scalar2=1e-8, op0=Alu.mult,
                    op1=Alu.max)
                nc.vector.reciprocal(
                    ppack[:, 4 * r + 1:4 * r + 2], ppack[:, 4 * r:4 * r + 1])
                tt = stp.tile([128, 1], f32, tag="tt")
                nc.vector.tensor_mul(
                    tt[:, :], mn[:, :], ppack[:, 4 * r + 1:4 * r + 2])
                # bq' = -mn*rs - 8 + 1536
                nc.vector.tensor_scalar(
                    out=ppack[:, 4 * r + 2:4 * r + 3], in0=tt[:, :],
                    scalar1=-1.0, scalar2=C16 - 8.0, op0=Alu.mult, op1=Alu.add)
                # alpha = mn + 8*scale (zero-point term; rides un-scaled path)
                t8 = stp.tile([128, 1], f32, tag="t8")
                nc.vector.tensor_scalar(
                    out=t8[:, :], in0=ppack[:, 4 * r:4 * r + 1],
                    scalar1=8.0, scalar2=None, op0=Alu.mult)
                nc.vector.tensor_add(
                    ppack[:, 4 * r + 3:4 * r + 4], t8[:, :], mn[:, :])

                # quantize+round: f16 cast of x*rs + (bq+1536) is the RNE
                nc.scalar.activation(
                    out=ya[:, :INT_F], in_=xt[:, :],
                    func=mybir.ActivationFunctionType.Identity,
                    bias=ppack[:, 4 * r + 2:4 * r + 3],
                    scale=ppack[:, 4 * r + 1:4 * r + 2])
                nc.vector.tensor_copy(
                    out=ya[:, INT_F:INT_F + 1],
                    in_=ppack[:, 4 * r + 3:4 * r + 4])
                # k-major transpose via DMA xbar
                nc.sync.dma_start_transpose(
                    out=qtb.rearrange("p (e t) -> p e t", e=KE + 1),
                    in_=ya[:, :])
                # de-bias fused into the fp8 copy (split DVE/ACT by parity)
                q8 = qt8.tile([128, KE, 128], f8)
                q8f = q8.rearrange("p e t -> p (e t)")
                if r % 2 == 0:
                    nc.vector.tensor_scalar(
                        out=q8f[:, :], in0=qtb[:, :INT_F], scalar1=-C16,
                        scalar2=None, op0=Alu.add)
                else:
                    nc.scalar.activation(
                        out=q8f[:, :], in_=qtb[:, :INT_F],
                        func=mybir.ActivationFunctionType.Identity,
                        bias=negc[:, :], scale=onec[:, :])
                return q8

            def consumer(r, q8):
                qtb = qtb_b[r % 2]
                t0 = r * 128
                for c in range(NCH):
                    o0 = c * CHW
                    p0 = ps0.tile([128, CHW], f32)
                    for e in range(KE // 2):
                        nc.tensor.matmul(
                            p0[:, :], q8[:, 2 * e:2 * e + 2, :],
                            wq_s[:, c, 2 * e:2 * e + 2, :],
                            start=(e == 0), stop=(e == KE // 2 - 1),
                            perf_mode=DR)
                    p1 = ps1.tile([128, CHW], f32)
                    for e in range(2):
                        nc.tensor.matmul(
                            p1[:, :], fpx8_s[:, e, :, t0:t0 + 128],
                            fpw8_s[:, e, :, o0:o0 + CHW],
                            start=(e == 0), stop=False, perf_mode=DR)
                    nc.tensor.matmul(
                        p1[:, :], qtb[0:1, INT_F:INT_F + 128],
                        redw_s[0:1, o0:o0 + CHW], start=False, stop=True)
                    m = mp.tile([128, CHW], f32)
                    nc.vector.tensor_mul(
                        m[:, :], p0[:, :], wsb_s[:, o0:o0 + CHW])
                    ot = outp.tile([128, CHW], f16)
                    nc.vector.affine_then_add(
                        out=ot[:, :], in0=m[:, :], in1=p1[:, :],
                        scale=ppack[:, 4 * r:4 * r + 1], bias=0.0)
                    nc.gpsimd.dma_start(
                        out=out_d[t0:t0 + 128, o0:o0 + CHW], in_=ot[:, :])

            # Software pipeline: producers run LA tiles ahead of consumers.
            # NOTE: LA must stay < the ya/qtb/q8 buffer count (2), else a
            # producer overwrites a buffer its consumer hasn't read yet.
            LA = 1
            made = {}
            for r in range(NT):
                if r == 0:
                    xt = xt0
                else:
                    xt = xin.tile([128, INT_F], f16, tag="xt")
                    nc.gpsimd.dma_start(
                        out=xt[:, :], in_=x_own[r * 128:(r + 1) * 128, :])
                made[r] = producer(r, xt)
                if r >= LA:
                    consumer(r - LA, made.pop(r - LA))
            for r in range(NT - LA, NT):
                consumer(r, made.pop(r))

    nc.finalize()
    return nc


def _get_program():
    global _PROGRAM
    if _PROGRAM is None:
        _PROGRAM = _build_program()
    return _PROGRAM


def _unpack_i4(w_packed):
    """(out, INT_F//2) uint8 -> (out, INT_F) int8; col 2k=low nibble, 2k+1=high."""
    lo = (w_packed & 0x0F).astype(np.int8)
    hi = ((w_packed >> 4) & 0x0F).astype(np.int8)
    lo = np.where(lo >= 8, lo - 16, lo)
    hi = np.where(hi >= 8, hi - 16, hi)
    w = np.empty((w_packed.shape[0], w_packed.shape[1] * 2), dtype=np.int8)
    w[:, 0::2] = lo
    w[:, 1::2] = hi
    return w


def _prep_inputs(x, int_weight, weights_scales, reduced_w, fp_weight, bias,
                 int_indices, fp_indices):
    import ml_dtypes
    f8np = ml_dtypes.float8_e4m3

    x2 = np.asarray(x, dtype=np.float16)[0]
    int_idx = np.asarray(int_indices).astype(np.int64)
    fp_idx = np.asarray(fp_indices).astype(np.int64)

    x_int = np.ascontiguousarray(x2[:, int_idx])            # (M, 3840) f16
    fp_xT = np.ascontiguousarray(x2[:, fp_idx].T)           # (256, M) f16

    w_int = _unpack_i4(np.asarray(int_weight))              # (OUT_F, 3840) int8
    wsc = np.asarray(weights_scales).astype(np.float16)     # (OUT_F, 1)
    redw = np.asarray(reduced_w).astype(np.float16)         # (1, OUT_F)
    fpW = np.asarray(fp_weight).astype(np.float16)          # (OUT_F, 256)
    b = np.asarray(bias).astype(np.float16)                 # (OUT_F,)

    # int weights: [p, chunk, ktile, out-in-chunk], value w[o, k], k=e*128+p
    wq = np.ascontiguousarray(
        w_int.T.reshape(KE, 128, NCH, CHW).transpose(1, 2, 0, 3)
    ).astype(f8np)

    # fp weights fp8, padded K=512: rows 0..255 fp features, 256 bias, rest 0
    fpw_aug = np.zeros((KPF, OUT_F), dtype=np.float32)
    fpw_aug[:FP_F] = fpW.T.astype(np.float32)
    fpw_aug[FP_F] = b.astype(np.float32)
    fpw8 = np.ascontiguousarray(
        fpw_aug.reshape(2, 2, 128, OUT_F).transpose(2, 0, 1, 3)
    ).astype(f8np)

    wsb = np.broadcast_to(wsc[:, 0][None, :], (128, OUT_F)).copy()
    redw_row = redw.copy()                                   # (1, OUT_F)

    in_maps = []
    for c in range(NCORES):
        tok = slice(c * TPC, (c + 1) * TPC)
        x_ownc = np.ascontiguousarray(x_int[tok])
        fpx_aug = np.zeros((KPF, TPC), dtype=np.float32)
        fpx_aug[:FP_F] = fp_xT[:, tok].astype(np.float32)
        fpx_aug[FP_F] = 1.0                                  # ones row (bias)
        fpx8 = np.ascontiguousarray(
            fpx_aug.reshape(2, 2, 128, TPC).transpose(2, 0, 1, 3)
        ).astype(f8np)
        in_maps.append({"x_own": x_ownc, "fpx8": fpx8, "wq": wq,
                        "fpw8": fpw8, "wsb": wsb, "redw": redw_row})
    return in_maps


def kernel(x, int_weight, weights_scales, reduced_w, fp_weight, bias,
           int_indices, fp_indices):
    global LAST_RESULTS
    from concourse.bass_utils import run_bass_kernel_spmd

    _ensure_ntff_hook()
    in_maps = _prep_inputs(x, int_weight, weights_scales, reduced_w,
                           fp_weight, bias, int_indices, fp_indices)
    nc = _get_program()
    res = run_bass_kernel_spmd(nc, in_maps, core_ids=list(range(NCORES)))
    LAST_RESULTS = res
    out = np.concatenate([res.results[c]["out"] for c in range(NCORES)],
                         axis=0)
    return out[None].astype(np.float16)


# revision 12
# speedup vs baseline: 2.2550x; 1.2253x over previous
"""MixedQLinear Trainium2 kernel — token-sharded (data-parallel) version.

Computation (per reference):
  x2 = x[0]                                  (M=4096, IN_F=4096) fp16
  int_x = x2[:, int_indices]                 (M, 3840)
  fp_x  = x2[:, fp_indices]                  (M, 256)
  per-token asym quant of int_x to int4:  scale=(mx-mn)/15, zero=mn
  q = round((int_x-zero)/scale) - 8          in [-8,7]
  out = scale*w_scale*(q @ w_int.T) + (zero+8*scale)*reduced_w + fp_x@fp_w.T + bias

Strategy: shard TOKENS across the 8 cores (512 each); every core holds the
FULL weight set in SBUF (int4 weights as fp8 ~15.7MB, fp path fp8, scales).
Each core quantizes only its own 4 token tiles — 8x less DVE/ACT/transpose
work than out-feature sharding, and no collective at all.

Device-side per core, per 128-token tile:
  - min/max stats (DVE trees, scratch carved from the ya/qtb buffers),
  - ACT emits ya = x*rs + (bq+1536) in f16: the f16 output cast IS the
    round-to-nearest-int (f16 ulp=1 on [1024,2048)), so no separate
    rounding pass; alpha = mn+8*scale rides as an extra column,
  - DMA xbar transpose (128 x 3968) -> k-major,
  - the -1536 de-bias is folded into the f16->fp8 copy (tensor_scalar),
  - per 512-wide out chunk (8 chunks): 15 fp8 DoubleRow matmuls (int path,
    K=3840) into p0; 2 fp8 DR matmuls (fp path, K=512: 256 fp features +
    ones-row bias + pad) plus the fp16 alpha x reduced_w rank-1 into p1,
  - combine: out = (p0*wscale)*scale_t + p1  (DVE mul + affine_then_add).

Host side: column gather, int4 unpack, fp8 casts, k-major weight layout.
"""

import os
import sys

import numpy as np

for _p in ("/opt/trn_rl_repo",):
    if _p not in sys.path and os.path.isdir(_p):
        sys.path.insert(0, _p)

TOKENS = 4096
IN_F = 4096
OUT_F = 4096
FP_F = 256
INT_F = IN_F - FP_F          # 3840
NCORES = 8
TPC = TOKENS // NCORES       # 512 tokens per core
NT = TPC // 128              # 4 token tiles per core
KE = INT_F // 128            # 30 int k-tiles
NCH = 8                      # out chunks
CHW = OUT_F // NCH           # 512 outs per chunk
KPF = 512                    # padded fp-path contraction (256 fp + bias + pad)
C16 = 1536.0                 # f16 magic: cast of (v+1536) to f16 rounds v to int

_PROGRAM = None
LAST_RESULTS = None


def _ensure_ntff_hook():
    """Install the axon NTFF profiling hook if the image's antenv lacks it.

    Best-effort: profiling only; compile/run work without it.
    """
    import contextlib
    import ctypes
    import types

    try:
        try:
            import antenv.axon_hooks as hooks_mod
        except ImportError:
            import antenv

            hooks_mod = types.ModuleType("antenv.axon_hooks")
            _holder = {}
            hooks_mod.set_axon_ntff_profile_hook = (
                lambda hook: _holder.__setitem__("hook", hook))
            hooks_mod.get_axon_ntff_profile_hook = (
                lambda: _holder.get("hook"))
            sys.modules["antenv.axon_hooks"] = hooks_mod
            antenv.axon_hooks = hooks_mod

        if hooks_mod.get_axon_ntff_profile_hook() is not None:
            return
        so_path = "/opt/axon/libaxon_pjrt.so"
        if not os.path.exists(so_path):
            return
        lib = ctypes.CDLL(so_path)
        if not hasattr(lib, "axon_start_nrt_profile"):
            return
        lib.axon_start_nrt_profile.argtypes = [
            ctypes.POINTER(ctypes.c_int64), ctypes.c_size_t]
        lib.axon_start_nrt_profile.restype = ctypes.c_int64
        lib.axon_stop_nrt_profile.argtypes = [ctypes.c_char_p]
        lib.axon_stop_nrt_profile.restype = ctypes.c_int64

        @contextlib.contextmanager
        def _hook(output_dir, device_ids):
            import jax

            jax.devices()
            if device_ids:
                ids = (ctypes.c_int64 * len(device_ids))(*device_ids)
                rc = lib.axon_start_nrt_profile(ids, len(device_ids))
            else:
                rc = lib.axon_start_nrt_profile(None, 0)
            if rc != 0:
                raise RuntimeError(f"axon_start_nrt_profile rc={rc}")
            try:
                yield
            finally:
                n = lib.axon_stop_nrt_profile(str(output_dir).encode())
                print(f"ntff profile: {n} file(s) written to {output_dir}")

        hooks_mod.set_axon_ntff_profile_hook(_hook)
    except Exception:
        pass


def _build_program():
    import concourse.mybir as mybir
    import concourse.tile as tile
    from concourse import bacc

    f16 = mybir.dt.float16
    f32 = mybir.dt.float32
    f8 = mybir.dt.float8e4
    Alu = mybir.AluOpType
    DR = mybir.MatmulPerfMode.DoubleRow

    nc = bacc.Bacc(None, target_bir_lowering=False)

    x_own = nc.dram_tensor("x_own", [TPC, INT_F], f16, kind="ExternalInput")
    fpx8_d = nc.dram_tensor("fpx8", [128, 2, TPC], f8, kind="ExternalInput")
    wq_d = nc.dram_tensor("wq", [128, NCH, KE, CHW], f8, kind="ExternalInput")
    fpw8_d = nc.dram_tensor("fpw8", [128, 2, OUT_F], f8,
                            kind="ExternalInput")
    wsb_d = nc.dram_tensor("wsb", [128, OUT_F], f16, kind="ExternalInput")
    redwb_d = nc.dram_tensor("redwb", [2, OUT_F], f16, kind="ExternalInput")
    out_d = nc.dram_tensor("out", [TPC, OUT_F], f16, kind="ExternalOutput")

    with tile.TileContext(nc) as tc:
        with tc.tile_pool(name="consts", bufs=1) as consts, \
             tc.tile_pool(name="xin", bufs=2) as xin, \
             tc.tile_pool(name="qt8", bufs=2) as qt8, \
             tc.tile_pool(name="stp", bufs=4) as stp, \
             tc.tile_pool(name="outp", bufs=2) as outp, \
             tc.tile_pool(name="mp", bufs=1) as mp, \
             tc.tile_pool(name="ps0", bufs=3, space="PSUM") as ps0, \
             tc.tile_pool(name="ps1", bufs=3, space="PSUM") as ps1:

            # A single DMA queue moves only ~30 GB/s, and each dma_start
            # costs ~0.6us of dispatch on the issuing engine. So: split the
            # first x tile across 4 queues (ready ~10us), keep the small
            # consts next, then fan the 15.7MB wq load over 16 sub-DMAs so
            # all queues pull weights at full aggregate HBM bandwidth.
            xt0 = xin.tile([128, INT_F], f16, tag="xt")
            for j in range(4):
                nc.gpsimd.dma_start(
                    out=xt0[32 * j:32 * (j + 1), :],
                    in_=x_own[32 * j:32 * (j + 1), :])

            fpx8_s = consts.tile([128, 2, TPC], f8)
            nc.gpsimd.dma_start(out=fpx8_s[:, :, :], in_=fpx8_d[:, :, :])
            wsb_s = consts.tile([128, OUT_F], f16)
            nc.gpsimd.dma_start(out=wsb_s[:, :], in_=wsb_d[:, :])
            redwb_s = consts.tile([2, OUT_F], f16)
            nc.gpsimd.dma_start(out=redwb_s[:, :], in_=redwb_d[:, :])
            fpw8_s = consts.tile([128, 2, OUT_F], f8)
            nc.gpsimd.dma_start(out=fpw8_s[:, :, :], in_=fpw8_d[:, :, :])
            xt1 = xin.tile([128, INT_F], f16, tag="xt")
            for j in range(2):
                nc.gpsimd.dma_start(
                    out=xt1[64 * j:64 * (j + 1), :],
                    in_=x_own[128 + 64 * j:128 + 64 * (j + 1), :])
            wq_s = consts.tile([128, NCH, KE, CHW], f8)
            for c in range(NCH):
                nc.gpsimd.dma_start(
                    out=wq_s[:, c, :KE // 2, :], in_=wq_d[:, c, :KE // 2, :])
                nc.gpsimd.dma_start(
                    out=wq_s[:, c, KE // 2:, :], in_=wq_d[:, c, KE // 2:, :])

            # Manual double-buffers for ya (pre-transpose, biased quant) and
            # qtb (post-transpose) so the pad columns can be zeroed once.
            ya_b = [consts.tile([128, KE * 128 + 128], f16, name=f"ya{i}",
                                tag=f"ya{i}") for i in range(2)]
            qtb_b = [consts.tile([128, (KE + 1) * 128], f16, name=f"qt{i}",
                                 tag=f"qt{i}") for i in range(2)]
            for i in range(2):
                nc.vector.memset(ya_b[i][:, INT_F + 1:INT_F + 2], 1.0)
                nc.vector.memset(ya_b[i][:, INT_F + 2:], 0.0)

            ppack = consts.tile([128, 4 * NT], f32)
            negc = consts.tile([128, 1], f32)
            nc.vector.memset(negc[:, :], -C16)
            onec = consts.tile([128, 1], f32)
            nc.vector.memset(onec[:, :], 1.0)

            def producer(r, xt):
                ya = ya_b[r % 2]
                qtb = qtb_b[r % 2]
                # min tree (scratch carved from ya; overwritten by ACT after)
                mn = stp.tile([128, 1], f32, tag="mn")
                mx = stp.tile([128, 1], f32, tag="mx")
                nc.vector.tensor_tensor(
                    out=ya[:, :1920], in0=xt[:, :1920], in1=xt[:, 1920:],
                    op=Alu.min)
                nc.vector.tensor_tensor(
                    out=ya[:, 1920:2880], in0=ya[:, :960], in1=ya[:, 960:1920],
                    op=Alu.min)
                nc.vector.tensor_reduce(
                    out=mn[:, :], in_=ya[:, 1920:2880],
                    axis=mybir.AxisListType.X, op=Alu.min)
                # max tree (scratch carved from qtb; overwritten by transpose)
                nc.vector.tensor_tensor(
                    out=qtb[:, :1920], in0=xt[:, :1920], in1=xt[:, 1920:],
                    op=Alu.max)
                nc.vector.tensor_tensor(
                    out=qtb[:, 1920:2880], in0=qtb[:, :960],
                    in1=qtb[:, 960:1920], op=Alu.max)
                nc.vector.tensor_reduce(
                    out=mx[:, :], in_=qtb[:, 1920:2880],
                    axis=mybir.AxisListType.X, op=Alu.max)
                # params: [scale, rs, bq+1536, alpha] at ppack[:, 4r:4r+4]
                d = stp.tile([128, 1], f32, tag="d")
                nc.vector.tensor_sub(d[:, :], mx[:, :], mn[:, :])
                nc.vector.tensor_scalar(
                    out=ppack[:, 4 * r:4 * r + 1], in0=d[:, :],
                    scalar1=1.0 / 15.0, scalar2=1e-8, op0=Alu.mult,
                    op1=Alu.max)
                nc.vector.reciprocal(
                    ppack[:, 4 * r + 1:4 * r + 2], ppack[:, 4 * r:4 * r + 1])
                tt = stp.tile([128, 1], f32, tag="tt")
                nc.vector.tensor_mul(
                    tt[:, :], mn[:, :], ppack[:, 4 * r + 1:4 * r + 2])
                # bq' = -mn*rs - 8 + 1536
                nc.vector.tensor_scalar(
                    out=ppack[:, 4 * r + 2:4 * r + 3], in0=tt[:, :],
                    scalar1=-1.0, scalar2=C16 - 8.0, op0=Alu.mult, op1=Alu.add)
                # alpha = mn + 8*scale (zero-point term; rides un-scaled path)
                t8 = stp.tile([128, 1], f32, tag="t8")
                nc.vector.tensor_scalar(
                    out=t8[:, :], in0=ppack[:, 4 * r:4 * r + 1],
                    scalar1=8.0, scalar2=None, op0=Alu.mult)
                nc.vector.tensor_add(
                    ppack[:, 4 * r + 3:4 * r + 4], t8[:, :], mn[:, :])

                # quantize+round: f16 cast of x*rs + (bq+1536) is the RNE
                nc.scalar.activation(
                    out=ya[:, :INT_F], in_=xt[:, :],
                    func=mybir.ActivationFunctionType.Identity,
                    bias=ppack[:, 4 * r + 2:4 * r + 3],
                    scale=ppack[:, 4 * r + 1:4 * r + 2])
                nc.vector.tensor_copy(
                    out=ya[:, INT_F:INT_F + 1],
                    in_=ppack[:, 4 * r + 3:4 * r + 4])
                # k-major transpose via DMA xbar
                nc.sync.dma_start_transpose(
                    out=qtb.rearrange("p (e t) -> p e t", e=KE + 1),
                    in_=ya[:, :])
                # de-bias fused into the fp8 copy (split DVE/ACT by parity)
                q8 = qt8.tile([128, KE, 128], f8)
                q8f = q8.rearrange("p e t -> p (e t)")
                if r % 2 == 0:
                    nc.vector.tensor_scalar(
                        out=q8f[:, :], in0=qtb[:, :INT_F], scalar1=-C16,
                        scalar2=None, op0=Alu.add)
                else:
                    nc.scalar.activation(
                        out=q8f[:, :], in_=qtb[:, :INT_F],
                        func=mybir.ActivationFunctionType.Identity,
                        bias=negc[:, :], scale=onec[:, :])
                return q8

            def consumer(r, q8):
                qtb = qtb_b[r % 2]
                t0 = r * 128
                for c in range(NCH):
                    o0 = c * CHW
                    p0 = ps0.tile([128, CHW], f32)
                    for e in range(KE // 2):
                        nc.tensor.matmul(
                            p0[:, :], q8[:, 2 * e:2 * e + 2, :],
                            wq_s[:, c, 2 * e:2 * e + 2, :],
                            start=(e == 0), stop=(e == KE // 2 - 1),
                            perf_mode=DR)
                    p1 = ps1.tile([128, CHW], f32)
                    nc.tensor.matmul(
                        p1[:, :], fpx8_s[:, :, t0:t0 + 128],
                        fpw8_s[:, :, o0:o0 + CHW],
                        start=True, stop=False, perf_mode=DR)
                    # K=2: [alpha; ones] x [reduced_w; bias] (ones rode the
                    # transpose as ya column INT_F+1)
                    nc.tensor.matmul(
                        p1[:, :], qtb[0:2, INT_F:INT_F + 128],
                        redwb_s[:, o0:o0 + CHW], start=False, stop=True)
                    m = mp.tile([128, CHW], f32)
                    nc.vector.tensor_mul(
                        m[:, :], p0[:, :], wsb_s[:, o0:o0 + CHW])
                    ot = outp.tile([128, CHW], f16)
                    nc.vector.affine_then_add(
                        out=ot[:, :], in0=m[:, :], in1=p1[:, :],
                        scale=ppack[:, 4 * r:4 * r + 1], bias=0.0)
                    nc.gpsimd.dma_start(
                        out=out_d[t0:t0 + 128, o0:o0 + CHW], in_=ot[:, :])

            # Software pipeline: producers run LA tiles ahead of consumers.
            # NOTE: LA must stay < the ya/qtb/q8 buffer count (2), else a
            # producer overwrites a buffer its consumer hasn't read yet.
            LA = 1
            made = {}
            for r in range(NT):
                if r == 0:
                    xt = xt0
                elif r == 1:
                    xt = xt1
                else:
                    xt = xin.tile([128, INT_F], f16, tag="xt")
                    nc.gpsimd.dma_start(
                        out=xt[:, :], in_=x_own[r * 128:(r + 1) * 128, :])
                made[r] = producer(r, xt)
                if r >= LA:
                    consumer(r - LA, made.pop(r - LA))
            for r in range(NT - LA, NT):
                consumer(r, made.pop(r))

    nc.finalize()
    return nc


def _get_program():
    global _PROGRAM
    if _PROGRAM is None:
        _PROGRAM = _build_program()
    return _PROGRAM


def _unpack_i4(w_packed):
    """(out, INT_F//2) uint8 -> (out, INT_F) int8; col 2k=low nibble, 2k+1=high."""
    lo = (w_packed & 0x0F).astype(np.int8)
    hi = ((w_packed >> 4) & 0x0F).astype(np.int8)
    lo = np.where(lo >= 8, lo - 16, lo)
    hi = np.where(hi >= 8, hi - 16, hi)
    w = np.empty((w_packed.shape[0], w_packed.shape[1] * 2), dtype=np.int8)
    w[:, 0::2] = lo
    w[:, 1::2] = hi
    return w


def _prep_inputs(x, int_weight, weights_scales, reduced_w, fp_weight, bias,
                 int_indices, fp_indices):
    import ml_dtypes
    f8np = ml_dtypes.float8_e4m3

    x2 = np.asarray(x, dtype=np.float16)[0]
    int_idx = np.asarray(int_indices).astype(np.int64)
    fp_idx = np.asarray(fp_indices).astype(np.int64)

    x_int = np.ascontiguousarray(x2[:, int_idx])            # (M, 3840) f16
    fp_xT = np.ascontiguousarray(x2[:, fp_idx].T)           # (256, M) f16

    w_int = _unpack_i4(np.asarray(int_weight))              # (OUT_F, 3840) int8
    wsc = np.asarray(weights_scales).astype(np.float16)     # (OUT_F, 1)
    redw = np.asarray(reduced_w).astype(np.float16)         # (1, OUT_F)
    fpW = np.asarray(fp_weight).astype(np.float16)          # (OUT_F, 256)
    b = np.asarray(bias).astype(np.float16)                 # (OUT_F,)

    # int weights: [p, chunk, ktile, out-in-chunk], value w[o, k], k=e*128+p
    wq = np.ascontiguousarray(
        w_int.T.reshape(KE, 128, NCH, CHW).transpose(1, 2, 0, 3)
    ).astype(f8np)

    # fp weights fp8, K=256 exactly: [p, i, o] holds fpW[o, k=i*128+p]
    fpw8 = np.ascontiguousarray(
        fpW.T.astype(np.float32).reshape(2, 128, OUT_F).transpose(1, 0, 2)
    ).astype(f8np)

    wsb = np.broadcast_to(wsc[:, 0][None, :], (128, OUT_F)).copy()
    # row 0: reduced_w (the alpha term), row 1: bias (multiplied by ones)
    redwb = np.ascontiguousarray(
        np.stack([redw[0].astype(np.float16), b], axis=0))   # (2, OUT_F)

    in_maps = []
    for c in range(NCORES):
        tok = slice(c * TPC, (c + 1) * TPC)
        x_ownc = np.ascontiguousarray(x_int[tok])
        fpx8 = np.ascontiguousarray(
            fp_xT[:, tok].astype(np.float32).reshape(2, 128, TPC)
            .transpose(1, 0, 2)).astype(f8np)
        in_maps.append({"x_own": x_ownc, "fpx8": fpx8, "wq": wq,
                        "fpw8": fpw8, "wsb": wsb, "redwb": redwb})
    return in_maps


def kernel(x, int_weight, weights_scales, reduced_w, fp_weight, bias,
           int_indices, fp_indices):
    global LAST_RESULTS
    from concourse.bass_utils import run_bass_kernel_spmd

    _ensure_ntff_hook()
    in_maps = _prep_inputs(x, int_weight, weights_scales, reduced_w,
                           fp_weight, bias, int_indices, fp_indices)
    nc = _get_program()
    res = run_bass_kernel_spmd(nc, in_maps, core_ids=list(range(NCORES)))
    LAST_RESULTS = res
    out = np.concatenate([res.results[c]["out"] for c in range(NCORES)],
                         axis=0)
    return out[None].astype(np.float16)
